# Initial kernel scaffold
#
"""Trainium2 Bass kernel for nn_CosineSimCausalTransformer (B=2, N=2048, D=512,
V=32000, 6 layers, 8 heads, cosine-sim causal attention, exact-gelu FFN).

Sharding (8 NeuronCores): 2 batch groups x 4 ranks. Core c = (batch=c//4,
rank r=c%4) owns token rows r::4 of its batch (512 rows) -- token-interleaved
so every core has an IDENTICAL causal block structure (true SPMD). Per layer
the normalized K (feature-major) and V (token-major) of the 4 ranks are
AllGather'ed within each batch group; attention runs over chunk-prefix
coverage with per-core diagonal masks supplied as input data.

All matmuls run in bf16 with fp32 PSUM accumulation. Softmax skips
max-subtraction (scores bounded to [-8, 8] by cosine-sim), folds 8/|q| into
q, and folds the 1/sum normalization into the p-transpose via a diag(1/d)
matmul.
"""
import os
import sys

sys.path.insert(0, "/opt/trn_rl_repo")

import numpy as np
import ml_dtypes

import concourse.bass as bass
import concourse.bacc as bacc_mod
import concourse.mybir as mybir
import concourse.tile as tile
from concourse.bass import ts
from concourse.masks import make_identity

F32 = mybir.dt.float32
BF16 = mybir.dt.bfloat16

B, N, D, V = 2, 2048, 512, 32000
DEPTH, H, DH, DFF = 6, 8, 64, 2048
SCALE = 8.0
RES_SCALE = float((2 * DEPTH) ** 0.25)
EPS_LN = 1e-5
TLOC = 512        # tokens per core
QT = 4            # 128-row token tiles per core
KO = 4            # D/128 contraction subtiles
CH = 4            # key chunks (ranks per group)
MO = DFF // 128   # 16
NCHUNK, NCOLS = 16, 2000   # Wlogits column streaming (16*2000 = 32000)
NSUB = 4, 500

AX = mybir.AxisListType.X
ALU = mybir.AluOpType
ACTF = mybir.ActivationFunctionType


def build_nc():
    nc = bacc_mod.Bacc()

    h0_p = nc.declare_dram_parameter("h0", [TLOC, D], F32, isOutput=False)
    mask_p = nc.declare_dram_parameter("masks", [CH, 128, 128], BF16, isOutput=False)
    wq_p = nc.declare_dram_parameter("wq", [DEPTH, D, D], BF16, isOutput=False)
    wk_p = nc.declare_dram_parameter("wk", [DEPTH, D, D], BF16, isOutput=False)
    wv_p = nc.declare_dram_parameter("wv", [DEPTH, D, D], BF16, isOutput=False)
    wo_p = nc.declare_dram_parameter("wo", [DEPTH, D, D], BF16, isOutput=False)
    w1_p = nc.declare_dram_parameter("w1", [DEPTH, D, DFF], BF16, isOutput=False)
    w2_p = nc.declare_dram_parameter("w2", [DEPTH, DFF, D], BF16, isOutput=False)
    ln1g_p = nc.declare_dram_parameter("ln1g", [DEPTH, D], F32, isOutput=False)
    ln1b_p = nc.declare_dram_parameter("ln1b", [DEPTH, D], F32, isOutput=False)
    ln2g_p = nc.declare_dram_parameter("ln2g", [DEPTH, D], F32, isOutput=False)
    ln2b_p = nc.declare_dram_parameter("ln2b", [DEPTH, D], F32, isOutput=False)
    wl_p = nc.declare_dram_parameter("wl", [D, V], BF16, isOutput=False)
    out_p = nc.declare_dram_parameter("out", [TLOC, V], BF16, isOutput=True)

    groups = [[0, 1, 2, 3], [4, 5, 6, 7]]

    def bcast_row(ap_1d, parts=128):
        # [D] dram AP -> [[0,parts], ...] broadcast read
        return bass.AP(tensor=ap_1d.tensor, offset=ap_1d.offset,
                       ap=[[0, parts]] + list(ap_1d.ap))

    with tile.TileContext(nc) as tc:
        import contextlib
        with contextlib.ExitStack() as top:
            const = top.enter_context(tc.tile_pool(name="const", bufs=1))
            hpool = top.enter_context(tc.tile_pool(name="hpool", bufs=1))
            hfpool = top.enter_context(tc.tile_pool(name="hfpool", bufs=1))

            ident = const.tile([128, 128], BF16)
            make_identity(nc, ident)
            masks_sb = const.tile([128, CH, 128], BF16)
            eps_t = const.tile([128, 1], F32)
            nc.vector.memset(eps_t, EPS_LN)
            nc.sync.dma_start(out=masks_sb, in_=mask_p[:].rearrange("c j m -> j c m"))

            h = hpool.tile([128, QT, D], F32)   # persistent token-major h
            nc.sync.dma_start(out=h, in_=h0_p[:].rearrange("(t p) d -> p t d", p=128))
            hf = hfpool.tile([128, KO, TLOC], BF16)  # feature-major bf16 h

            def transpose_to_fm(src_f32):
                """src [128, QT, D] f32 -> hf [128, KO, TLOC] bf16 (hf[d%128, d//128, tok])"""
                hb = hfpool.tile([128, QT, D], BF16, tag="hcast")
                for t in range(QT):
                    nc.scalar.copy(hb[:, t], src_f32[:, t])
                for t in range(QT):
                    tp = tpsum.tile([128, KO, 128], BF16, tag="tgrp")
                    for ko in range(KO):
                        nc.tensor.transpose(tp[:, ko], hb[:, t, ts(ko, 128)], ident)
                    nc.vector.tensor_copy(hf[:, :, ts(t, 128)], tp)

            with contextlib.ExitStack() as lay:
                wpool = lay.enter_context(tc.tile_pool(name="wpool", bufs=2))
                wbig = lay.enter_context(tc.tile_pool(name="wbig", bufs=1))
                gbpool = lay.enter_context(tc.tile_pool(name="gbpool", bufs=1))
                kvpool = lay.enter_context(tc.tile_pool(name="kvpool", bufs=1))
                qpool = lay.enter_context(tc.tile_pool(name="qpool", bufs=1))
                ppool = lay.enter_context(tc.tile_pool(name="ppool", bufs=2))
                stpool = lay.enter_context(tc.tile_pool(name="stpool", bufs=2))
                smalls = lay.enter_context(tc.tile_pool(name="smalls", bufs=4))
                ofpool = lay.enter_context(tc.tile_pool(name="ofpool", bufs=1))
                gfpool = lay.enter_context(tc.tile_pool(name="gfpool", bufs=1))
                dram = lay.enter_context(tc.tile_pool(name="dram", bufs=2, space="DRAM"))

                cpsum = lay.enter_context(tc.tile_pool(name="cpsum", bufs=2, space="PSUM"))
                spsum = lay.enter_context(tc.tile_pool(name="spsum", bufs=3, space="PSUM"))
                tpsum = lay.enter_context(tc.tile_pool(name="tpsum", bufs=2, space="PSUM"))
                upsum = lay.enter_context(tc.tile_pool(name="upsum", bufs=1, space="PSUM"))

                for li in range(DEPTH):
                    # ---- weights ----
                    wq = wpool.tile([128, KO, D], BF16, tag="wq")
                    wk = wpool.tile([128, KO, D], BF16, tag="wk")
                    wv = wpool.tile([128, KO, D], BF16, tag="wv")
                    wo = wpool.tile([128, KO, D], BF16, tag="wo")
                    for wt, wp in ((wq, wq_p), (wk, wk_p), (wv, wv_p), (wo, wo_p)):
                        nc.sync.dma_start(
                            out=wt, in_=wp[li].rearrange("(ko ki) f -> ki ko f", ki=128))
                    w1 = wbig.tile([128, KO, DFF], BF16, tag="w1")
                    nc.sync.dma_start(
                        out=w1, in_=w1_p[li].rearrange("(ko ki) f -> ki ko f", ki=128))
                    w2 = wbig.tile([128, MO, D], BF16, tag="w2")
                    nc.sync.dma_start(
                        out=w2, in_=w2_p[li].rearrange("(mo ki) f -> ki mo f", ki=128))
                    gb1 = gbpool.tile([128, 2, D], F32, tag="gb1")
                    nc.sync.dma_start(out=gb1[:, 0], in_=bcast_row(ln1g_p[li]))
                    nc.sync.dma_start(out=gb1[:, 1], in_=bcast_row(ln1b_p[li]))
                    gb2 = gbpool.tile([128, 2, D], F32, tag="gb2")
                    nc.sync.dma_start(out=gb2[:, 0], in_=bcast_row(ln2g_p[li]))
                    nc.sync.dma_start(out=gb2[:, 1], in_=bcast_row(ln2b_p[li]))

                    # ---- h -> feature-major bf16 ----
                    transpose_to_fm(h)

                    # ---- k (token-major), normalize, transpose, AG1 ----
                    k_local = dram.tile([TLOC, D], BF16, tag="k_local")
                    v_local = dram.tile([TLOC, D], BF16, tag="v_local")
                    knf_loc = qpool.tile([128, CH, TLOC], BF16, tag="knf_loc")
                    v_loc = qpool.tile([128, QT, D], BF16, tag="v_loc")
                    for t in range(QT):
                        kp = cpsum.tile([128, D], F32, tag="chain")
                        for ko in range(KO):
                            nc.tensor.matmul(kp, hf[:, ko, ts(t, 128)], wk[:, ko],
                                             start=ko == 0, stop=ko == KO - 1)
                        sq = smalls.tile([128, D], F32, tag="sq", bufs=2)
                        nc.scalar.square(sq, kp)
                        ssq = smalls.tile([128, H], F32, tag="ssq")
                        nc.vector.reduce_sum(ssq, sq.rearrange("p (h d) -> p h d", h=H),
                                             axis=AX)
                        nrm = smalls.tile([128, H], F32, tag="nrm")
                        nc.scalar.activation(nrm, ssq, ACTF.Sqrt)
                        nc.vector.tensor_scalar_max(nrm, nrm, 1e-12)
                        rk = smalls.tile([128, H], F32, tag="rk")
                        nc.vector.reciprocal(rk, nrm)
                        kn = qpool.tile([128, H, DH], BF16, tag="kn_tm")
                        for hd in range(H):
                            nc.vector.tensor_scalar_mul(
                                kn[:, hd], kp[:, ts(hd, DH)], rk[:, hd:hd + 1])
                        tp = tpsum.tile([64, H, 128], BF16, tag="tgrp",
                                        name="tpk")
                        for hd in range(H):
                            slot = (hd % 2) * 4 + hd // 2
                            nc.tensor.transpose(tp[:, slot], kn[:, hd], ident)
                        nc.vector.tensor_copy(
                            knf_loc[0:64, :, ts(t, 128)], tp[:, 0:4])
                        nc.vector.tensor_copy(
                            knf_loc[64:128, :, ts(t, 128)], tp[:, 4:8])
                    nc.sync.dma_start(
                        out=k_local[:].rearrange("(hp p) l -> p hp l", p=128),
                        in_=knf_loc)
                    k_full = dram.tile([CH * TLOC, D], BF16, tag="k_full")
                    nc.gpsimd.collective_compute(
                        "AllGather", ALU.bypass, replica_groups=groups,
                        ins=[k_local.opt()], outs=[k_full.opt()])

                    # ---- v (token-major), AG2 ----
                    for t in range(QT):
                        vp = cpsum.tile([128, D], F32, tag="chain")
                        for ko in range(KO):
                            nc.tensor.matmul(vp, hf[:, ko, ts(t, 128)], wv[:, ko],
                                             start=ko == 0, stop=ko == KO - 1)
                        nc.scalar.copy(v_loc[:, t], vp)
                    nc.sync.dma_start(
                        out=v_local[:].rearrange("(t p) f -> p t f", p=128),
                        in_=v_loc)
                    v_full = dram.tile([CH * TLOC, D], BF16, tag="v_full")
                    nc.gpsimd.collective_compute(
                        "AllGather", ALU.bypass, replica_groups=groups,
                        ins=[v_local.opt()], outs=[v_full.opt()])

                    # ---- q (token-major) + 8/|q| fold + transpose to fm ----
                    qnf = qpool.tile([128, CH, TLOC], BF16, tag="qnf")
                    for t in range(QT):
                        qp = cpsum.tile([128, D], F32, tag="chain")
                        for ko in range(KO):
                            nc.tensor.matmul(qp, hf[:, ko, ts(t, 128)], wq[:, ko],
                                             start=ko == 0, stop=ko == KO - 1)
                        sq = smalls.tile([128, D], F32, tag="sq", bufs=2)
                        nc.scalar.square(sq, qp)
                        ssq = smalls.tile([128, H], F32, tag="ssq")
                        nc.vector.reduce_sum(ssq, sq.rearrange("p (h d) -> p h d", h=H),
                                             axis=AX)
                        nrm = smalls.tile([128, H], F32, tag="nrm")
                        nc.scalar.activation(nrm, ssq, ACTF.Sqrt)
                        nc.vector.tensor_scalar_max(nrm, nrm, 1e-12)
                        rq = smalls.tile([128, H], F32, tag="rk")
                        nc.vector.reciprocal(rq, nrm)
                        qn = qpool.tile([128, H, DH], BF16, tag="qn_tm")
                        for hd in range(H):
                            nc.vector.tensor_scalar(
                                qn[:, hd], qp[:, ts(hd, DH)], rq[:, hd:hd + 1],
                                SCALE, ALU.mult, ALU.mult)
                        tp = tpsum.tile([64, H, 128], BF16, tag="tgrp",
                                        name="tpq")
                        for hd in range(H):
                            slot = (hd % 2) * 4 + hd // 2
                            nc.tensor.transpose(tp[:, slot], qn[:, hd], ident)
                        nc.vector.tensor_copy(
                            qnf[0:64, :, ts(t, 128)], tp[:, 0:4])
                        nc.vector.tensor_copy(
                            qnf[64:128, :, ts(t, 128)], tp[:, 4:8])

                    # ---- AG readback ----
                    knf = kvpool.tile([128, CH, CH, TLOC], BF16, tag="knf")  # [2h, hp, c, l]
                    vfull = kvpool.tile([128, CH, QT, D], BF16, tag="vfull")  # [p, c, t, hd*dh]
                    for c in range(CH):
                        nc.sync.dma_start(
                            out=knf[:, :, c],
                            in_=k_full[c * TLOC:(c + 1) * TLOC]
                            .rearrange("(hp p) l -> p hp l", p=128))
                        nc.sync.dma_start(
                            out=vfull[:, c],
                            in_=v_full[c * TLOC:(c + 1) * TLOC]
                            .rearrange("(t p) f -> p t f", p=128))

                    # ---- attention ----
                    of = ofpool.tile([128, KO, TLOC], BF16, tag="of")
                    for hd in range(H):
                        hp, half = hd // 2, hd % 2
                        po = 64 * half
                        strips = [stpool.tile([128, QT, TLOC], BF16, tag=f"strip{c}",
                                              name=f"strip{c}")
                                  for c in range(CH)]
                        for t in range(QT):
                            span = (t + 1) * 128
                            pb = ppool.tile([128, CH, TLOC], BF16, tag="p")
                            for c in range(CH):
                                sp = spsum.tile([128, TLOC], F32, tag="s")
                                nc.tensor.matmul(
                                    sp[:, :span],
                                    qnf[po:po + 64, hp, ts(t, 128)],
                                    knf[po:po + 64, hp, c, :span],
                                    start=True, stop=True)
                                nc.scalar.activation(pb[:, c, :span], sp[:, :span],
                                                     ACTF.Exp)
                            nc.vector.tensor_mul(
                                pb[:, :, ts(t, 128)], pb[:, :, ts(t, 128)],
                                masks_sb)
                            dparts = smalls.tile([128, CH], F32, tag="dparts")
                            nc.vector.reduce_sum(dparts, pb[:, :, :span], axis=AX)
                            dsum = smalls.tile([128, 1], F32, tag="dsum")
                            nc.vector.reduce_sum(dsum, dparts, axis=AX)
                            rinv = smalls.tile([128, 1], F32, tag="rinv")
                            nc.vector.reciprocal(rinv, dsum)
                            nc.vector.tensor_scalar_mul(
                                pb[:, :, :span], pb[:, :, :span], rinv)
                            for c in range(CH):
                                for idx in range(t + 1):
                                    nc.sync.dma_start_transpose(
                                        strips[c][:, idx, ts(t, 128)],
                                        pb[:, c, ts(idx, 128)])
                        u = upsum.tile([64, TLOC], F32, tag="u")
                        first = True
                        for idx in range(QT):
                            for c in range(CH):
                                nc.tensor.matmul(
                                    u[:, idx * 128:],
                                    vfull[:, c, idx, ts(hd, DH)],
                                    strips[c][:, idx, idx * 128:],
                                    start=first, stop=(idx == QT - 1 and c == CH - 1),
                                    skip_group_check=True)
                                first = False
                        nc.vector.tensor_copy(of[po:po + 64, hp], u)

                    # ---- Wo + residual + LN1 ----
                    for t in range(QT):
                        ap_ = cpsum.tile([128, D], F32, tag="chain")
                        for ko in range(KO):
                            nc.tensor.matmul(ap_, of[:, ko, ts(t, 128)], wo[:, ko],
                                             start=ko == 0, stop=ko == KO - 1)
                        nc.vector.tensor_scalar_mul(h[:, t], h[:, t], RES_SCALE)
                        nc.vector.tensor_add(h[:, t], h[:, t], ap_)
                        st = smalls.tile([128, 6], F32, tag="st")
                        nc.vector.bn_stats(st, h[:, t])
                        mv = smalls.tile([128, 2], F32, tag="mv")
                        nc.vector.bn_aggr(mv, st)
                        sd = smalls.tile([128, 1], F32, tag="sd")
                        nc.scalar.activation(sd, mv[:, 1:2], ACTF.Sqrt, bias=eps_t)
                        rstd = smalls.tile([128, 1], F32, tag="rstd")
                        nc.vector.reciprocal(rstd, sd)
                        nc.vector.tensor_scalar(h[:, t], h[:, t], mv[:, 0:1], rstd,
                                                ALU.subtract, ALU.mult)
                        nc.vector.tensor_mul(h[:, t], h[:, t], gb1[:, 0])
                        nc.vector.tensor_add(h[:, t], h[:, t], gb1[:, 1])

                    # ---- FFN ----
                    transpose_to_fm(h)
                    gf = gfpool.tile([128, MO, TLOC], BF16, tag="gf")
                    for mo in range(MO):
                        up = cpsum.tile([128, TLOC], F32, tag="chain")
                        for ko in range(KO):
                            nc.tensor.matmul(up, w1[:, ko, ts(mo, 128)],
                                             hf[:, ko], start=ko == 0, stop=ko == KO - 1)
                        nc.scalar.activation(gf[:, mo], up, ACTF.Gelu)
                    for t in range(QT):
                        fp = cpsum.tile([128, D], F32, tag="chain")
                        for mo in range(MO):
                            nc.tensor.matmul(fp, gf[:, mo, ts(t, 128)], w2[:, mo],
                                             start=mo == 0, stop=mo == MO - 1)
                        nc.vector.tensor_scalar_mul(h[:, t], h[:, t], RES_SCALE)
                        nc.vector.tensor_add(h[:, t], h[:, t], fp)
                        st = smalls.tile([128, 6], F32, tag="st")
                        nc.vector.bn_stats(st, h[:, t])
                        mv = smalls.tile([128, 2], F32, tag="mv")
                        nc.vector.bn_aggr(mv, st)
                        sd = smalls.tile([128, 1], F32, tag="sd")
                        nc.scalar.activation(sd, mv[:, 1:2], ACTF.Sqrt, bias=eps_t)
                        rstd = smalls.tile([128, 1], F32, tag="rstd")
                        nc.vector.reciprocal(rstd, sd)
                        nc.vector.tensor_scalar(h[:, t], h[:, t], mv[:, 0:1], rstd,
                                                ALU.subtract, ALU.mult)
                        nc.vector.tensor_mul(h[:, t], h[:, t], gb2[:, 0])
                        nc.vector.tensor_add(h[:, t], h[:, t], gb2[:, 1])

            # ---- logits ----
            with contextlib.ExitStack() as lg:
                wlpool = lg.enter_context(tc.tile_pool(name="wlpool", bufs=3))
                zsb = lg.enter_context(tc.tile_pool(name="zsb", bufs=4))
                zpsum = lg.enter_context(tc.tile_pool(name="zpsum", bufs=6, space="PSUM"))
                tpsum = lg.enter_context(tc.tile_pool(name="tps2", bufs=2, space="PSUM"))
                transpose_to_fm(h)
                nsub, ncols = NSUB
                for chk in range(NCHUNK):
                    wl = wlpool.tile([128, KO, NCOLS], BF16, tag="wl")
                    half_c = NCOLS // 2
                    for hh in range(2):
                        nc.sync.dma_start(
                            out=wl[:, :, hh * half_c:(hh + 1) * half_c],
                            in_=wl_p[:, chk * NCOLS + hh * half_c:
                                     chk * NCOLS + (hh + 1) * half_c]
                            .rearrange("(ko ki) f -> ki ko f", ki=128))
                    for t in range(QT):
                        for nn in range(nsub):
                            zp = zpsum.tile([128, ncols], F32, tag="z")
                            for ko in range(KO):
                                nc.tensor.matmul(
                                    zp, hf[:, ko, ts(t, 128)],
                                    wl[:, ko, ts(nn, ncols)],
                                    start=ko == 0, stop=ko == KO - 1)
                            zs = zsb.tile([128, ncols], BF16, tag="zs")
                            if (t * nsub + nn) % 2 == 0:
                                nc.scalar.copy(zs, zp)
                            else:
                                nc.vector.tensor_copy(zs, zp)
                            nc.sync.dma_start(
                                out=out_p[ts(t, 128),
                                          chk * NCOLS + nn * ncols:
                                          chk * NCOLS + (nn + 1) * ncols],
                                in_=zs)
    if not nc.is_finalized():
        nc.finalize()
    return nc


_CACHE = {}


def _prep(inputs):
    bf = lambda a: np.ascontiguousarray(np.asarray(a)).astype(ml_dtypes.bfloat16)
    f32 = lambda a: np.ascontiguousarray(np.asarray(a, dtype=np.float32))
    x = np.asarray(inputs["x"])
    h0 = np.asarray(inputs["token_emb"])[x] + np.asarray(inputs["pos_emb"])[:N]
    h0 = h0.astype(np.float32)

    shared = dict(
        wq=bf(inputs["Wq"]), wk=bf(inputs["Wk"]), wv=bf(inputs["Wv"]),
        wo=bf(inputs["Wo"]), w1=bf(inputs["W1"]), w2=bf(inputs["W2"]),
        ln1g=f32(inputs["ln1_g"]), ln1b=f32(inputs["ln1_b"]),
        ln2g=f32(inputs["ln2_g"]), ln2b=f32(inputs["ln2_b"]),
        wl=bf(inputs["Wlogits"]),
    )
    j = np.arange(128)[:, None]
    m = np.arange(128)[None, :]
    in_maps = []
    for core in range(8):
        b, r = core // 4, core % 4
        masks = np.stack([
            np.where(4 * m + c <= 4 * j + r, 1.0, 0.0)
            for c in range(CH)]).astype(ml_dtypes.bfloat16)
        in_maps.append(dict(shared, h0=np.ascontiguousarray(h0[b, r::4]),
                            masks=masks))
    return in_maps


def _run(inputs, trace=False, **kw):
    from concourse.bass_utils import run_bass_kernel_spmd
    if "nc" not in _CACHE:
        _CACHE["nc"] = build_nc()
    nc = _CACHE["nc"]
    in_maps = _prep(inputs)
    res = run_bass_kernel_spmd(nc, in_maps, core_ids=list(range(8)),
                               trace=trace, **kw)
    out = np.zeros((B, N, V), np.float32)
    for core in range(8):
        b, r = core // 4, core % 4
        out[b, r::4] = res.results[core]["out"].astype(np.float32)
    return out, res


def kernel(**inputs):
    return _run(inputs, trace=False)[0]



# revision 13
# speedup vs baseline: 1.8314x; 1.8314x over previous
"""Trainium2 Bass kernel for nn_CosineSimCausalTransformer (B=2, N=2048, D=512,
V=32000, 6 layers, 8 heads, cosine-sim causal attention, exact-gelu FFN).

Sharding (8 NeuronCores): 2 batch groups x 4 ranks. Core c = (batch=c//4,
rank r=c%4) owns token rows r::4 of its batch (512 rows) -- token-interleaved
so every core has an IDENTICAL causal block structure (true SPMD). Per layer
the normalized K (feature-major) and V (token-major) of the 4 ranks are
AllGather'ed (single fused k+v collective) within each batch group.

Attention is transpose-free: scores are computed key-major (s^T = K_n Q_n^T),
exp'd straight into SBUF as the P^T operand of the P.V matmul (V stationary).
Softmax denominators come from a concurrent M=1 ones-matmul into a spare PSUM
col-group; 1/d is broadcast to 64 lanes with a K=1 PE matmul and folded into
the PSUM->SBUF copy of the attention output. Per-head lane alignment is kept
by swapping the 64-row halves of each head pair (Wo rows pre-swapped on host).

All matmuls run in bf16 with fp32 PSUM accumulation. Softmax skips
max-subtraction (scores bounded to [-8, 8] by cosine-sim) and folds 8/|q|
into q.
"""
import os
import sys

sys.path.insert(0, "/opt/trn_rl_repo")

import numpy as np
import ml_dtypes

import concourse.bass as bass
import concourse.bacc as bacc_mod
import concourse.mybir as mybir
import concourse.tile as tile
from concourse.bass import ts
from concourse.masks import make_identity

F32 = mybir.dt.float32
BF16 = mybir.dt.bfloat16

B, N, D, V = 2, 2048, 512, 32000
DEPTH, H, DH, DFF = 6, 8, 64, 2048
SCALE = 8.0
RES_SCALE = float((2 * DEPTH) ** 0.25)
EPS_LN = 1e-5
TLOC = 512        # tokens per core
QT = 4            # 128-row token tiles per core
KO = 4            # D/128 contraction subtiles
CH = 4            # key chunks (ranks per group)
MO = DFF // 128   # 16
NCHUNK, NCOLS = 16, 2000   # Wlogits column streaming (16*2000 = 32000)
NSUB = 4, 500

AX = mybir.AxisListType.X
ALU = mybir.AluOpType
ACTF = mybir.ActivationFunctionType


def build_nc():
    nc = bacc_mod.Bacc()

    h0_p = nc.declare_dram_parameter("h0", [TLOC, D], F32, isOutput=False)
    mask_p = nc.declare_dram_parameter("masks", [CH, 128, 128], BF16, isOutput=False)
    wq_p = nc.declare_dram_parameter("wq", [DEPTH, D, D], BF16, isOutput=False)
    wk_p = nc.declare_dram_parameter("wk", [DEPTH, D, D], BF16, isOutput=False)
    wv_p = nc.declare_dram_parameter("wv", [DEPTH, D, D], BF16, isOutput=False)
    wo_p = nc.declare_dram_parameter("wo", [DEPTH, D, D], BF16, isOutput=False)
    w1_p = nc.declare_dram_parameter("w1", [DEPTH, D, DFF], BF16, isOutput=False)
    w2_p = nc.declare_dram_parameter("w2", [DEPTH, DFF, D], BF16, isOutput=False)
    ln1g_p = nc.declare_dram_parameter("ln1g", [DEPTH, D], F32, isOutput=False)
    ln1b_p = nc.declare_dram_parameter("ln1b", [DEPTH, D], F32, isOutput=False)
    ln2g_p = nc.declare_dram_parameter("ln2g", [DEPTH, D], F32, isOutput=False)
    ln2b_p = nc.declare_dram_parameter("ln2b", [DEPTH, D], F32, isOutput=False)
    wl_p = nc.declare_dram_parameter("wl", [D, V], BF16, isOutput=False)
    out_p = nc.declare_dram_parameter("out", [TLOC, V], BF16, isOutput=True)

    groups = [[0, 1, 2, 3], [4, 5, 6, 7]]

    def bcast_row(ap_1d, parts=128):
        # [D] dram AP -> [[0,parts], ...] broadcast read
        return bass.AP(tensor=ap_1d.tensor, offset=ap_1d.offset,
                       ap=[[0, parts]] + list(ap_1d.ap))

    with tile.TileContext(nc) as tc:
        import contextlib
        with contextlib.ExitStack() as top:
            const = top.enter_context(tc.tile_pool(name="const", bufs=1))
            hpool = top.enter_context(tc.tile_pool(name="hpool", bufs=1))
            hfpool = top.enter_context(tc.tile_pool(name="hfpool", bufs=1))

            ident = const.tile([128, 128], BF16, name="ident")
            make_identity(nc, ident)
            identf = const.tile([128, 128], F32, name="identf")
            make_identity(nc, identf)
            masks_sb = const.tile([128, CH, 128], BF16, name="masks_sb")
            eps_t = const.tile([128, 1], F32, name="eps_t")
            nc.vector.memset(eps_t, EPS_LN)
            ones1 = const.tile([128, 1], BF16, name="ones1")
            nc.vector.memset(ones1, 1.0)
            e64 = const.tile([128, 64], F32, name="e64")
            nc.vector.memset(e64, 1.0)
            nc.sync.dma_start(out=masks_sb, in_=mask_p[:].rearrange("c j m -> j c m"))

            h = hpool.tile([128, QT, D], F32, name="h")   # persistent token-major h
            nc.sync.dma_start(out=h, in_=h0_p[:].rearrange("(t p) d -> p t d", p=128))
            hf = hfpool.tile([128, KO, TLOC], BF16, name="hf")  # feature-major bf16 h

            def transpose_to_fm(src_f32):
                """src [128, QT, D] f32 -> hf [128, KO, TLOC] bf16 (hf[d%128, d//128, tok])

                f32 PE transpose; the PSUM->SBUF copy performs the bf16 cast."""
                for t in range(QT):
                    tp = tpsum.tile([128, KO, 128], F32, tag="tgrp", name="tp")
                    for ko in range(KO):
                        nc.tensor.transpose(tp[:, ko], src_f32[:, t, ts(ko, 128)],
                                            identf)
                    nc.vector.tensor_copy(hf[:, :, ts(t, 128)], tp)

            with contextlib.ExitStack() as lay:
                wpool = lay.enter_context(tc.tile_pool(name="wpool", bufs=2))
                wbig = lay.enter_context(tc.tile_pool(name="wbig", bufs=1))
                gbpool = lay.enter_context(tc.tile_pool(name="gbpool", bufs=1))
                kvpool = lay.enter_context(tc.tile_pool(name="kvpool", bufs=1))
                qpool = lay.enter_context(tc.tile_pool(name="qpool", bufs=1))
                ppool = lay.enter_context(tc.tile_pool(name="ppool", bufs=4))
                smalls = lay.enter_context(tc.tile_pool(name="smalls", bufs=4))
                ofpool = lay.enter_context(tc.tile_pool(name="ofpool", bufs=1))
                gfpool = lay.enter_context(tc.tile_pool(name="gfpool", bufs=1))
                dram = lay.enter_context(tc.tile_pool(name="dram", bufs=2, space="DRAM"))

                cpsum = lay.enter_context(tc.tile_pool(name="cpsum", bufs=2, space="PSUM"))
                spsum = lay.enter_context(tc.tile_pool(name="spsum", bufs=2, space="PSUM"))
                tpsum = lay.enter_context(tc.tile_pool(name="tpsum", bufs=1, space="PSUM"))
                apsum = lay.enter_context(tc.tile_pool(name="apsum", bufs=1, space="PSUM"))

                for li in range(DEPTH):
                    # ---- weights ----
                    wq = wpool.tile([128, KO, D], BF16, tag="wq", name="wq")
                    wk = wpool.tile([128, KO, D], BF16, tag="wk", name="wk")
                    wv = wpool.tile([128, KO, D], BF16, tag="wv", name="wv")
                    wo = wpool.tile([128, KO, D], BF16, tag="wo", name="wo")
                    for wt, wp in ((wq, wq_p), (wk, wk_p), (wv, wv_p), (wo, wo_p)):
                        nc.sync.dma_start(
                            out=wt, in_=wp[li].rearrange("(ko ki) f -> ki ko f", ki=128))
                    w1 = wbig.tile([128, KO, DFF], BF16, tag="w1", name="w1")
                    nc.sync.dma_start(
                        out=w1, in_=w1_p[li].rearrange("(ko ki) f -> ki ko f", ki=128))
                    w2 = wbig.tile([128, MO, D], BF16, tag="w2", name="w2")
                    nc.sync.dma_start(
                        out=w2, in_=w2_p[li].rearrange("(mo ki) f -> ki mo f", ki=128))
                    gb1 = gbpool.tile([128, 2, D], F32, tag="gb1", name="gb1")
                    nc.sync.dma_start(out=gb1[:, 0], in_=bcast_row(ln1g_p[li]))
                    nc.sync.dma_start(out=gb1[:, 1], in_=bcast_row(ln1b_p[li]))
                    gb2 = gbpool.tile([128, 2, D], F32, tag="gb2", name="gb2")
                    nc.sync.dma_start(out=gb2[:, 0], in_=bcast_row(ln2g_p[li]))
                    nc.sync.dma_start(out=gb2[:, 1], in_=bcast_row(ln2b_p[li]))

                    # ---- h -> feature-major bf16 ----
                    transpose_to_fm(h)

                    # ---- k (token-major), normalize, transpose to fm ----
                    k_local = dram.tile([TLOC, D], BF16, tag="k_local",
                                        name="k_local")
                    v_local = dram.tile([TLOC, D], BF16, tag="v_local",
                                        name="v_local")
                    knf_loc = qpool.tile([128, CH, TLOC], BF16, tag="knf_loc",
                                         name="knf_loc")
                    v_loc = qpool.tile([128, QT, D], BF16, tag="v_loc", name="v_loc")
                    for t in range(QT):
                        kp = cpsum.tile([128, D], F32, tag="chain", name="kp")
                        for ko in range(KO):
                            nc.tensor.matmul(kp, hf[:, ko, ts(t, 128)], wk[:, ko],
                                             start=ko == 0, stop=ko == KO - 1)
                        sq = smalls.tile([128, D], F32, tag="sq", bufs=2, name="sq")
                        nc.scalar.square(sq, kp)
                        ssq = smalls.tile([128, H], F32, tag="ssq", name="ssq")
                        nc.vector.reduce_sum(ssq, sq.rearrange("p (h d) -> p h d", h=H),
                                             axis=AX)
                        nrm = smalls.tile([128, H], F32, tag="nrm", name="nrm")
                        nc.scalar.activation(nrm, ssq, ACTF.Sqrt)
                        nc.vector.tensor_scalar_max(nrm, nrm, 1e-12)
                        rk = smalls.tile([128, H], F32, tag="rk", name="rk")
                        nc.vector.reciprocal(rk, nrm)
                        kn = qpool.tile([128, H, DH], BF16, tag="kn_tm", name="kn")
                        for hd in range(H):
                            nc.vector.tensor_scalar_mul(
                                kn[:, hd], kp[:, ts(hd, DH)], rk[:, hd:hd + 1])
                        tp = tpsum.tile([64, H, 128], BF16, tag="tgrp", name="tpk")
                        for hd in range(H):
                            slot = (hd % 2) * 4 + hd // 2
                            nc.tensor.transpose(tp[:, slot], kn[:, hd], ident)
                        nc.vector.tensor_copy(
                            knf_loc[0:64, :, ts(t, 128)], tp[:, 0:4])
                        nc.vector.tensor_copy(
                            knf_loc[64:128, :, ts(t, 128)], tp[:, 4:8])
                    nc.sync.dma_start(
                        out=k_local[:].rearrange("(hp p) l -> p hp l", p=128),
                        in_=knf_loc)
                    k_full = dram.tile([CH * TLOC, D], BF16, tag="k_full",
                                       name="k_full")
                    nc.gpsimd.collective_compute(
                        "AllGather", ALU.bypass, replica_groups=groups,
                        ins=[k_local.opt()], outs=[k_full.opt()])

                    # ---- v (token-major) ----
                    for t in range(QT):
                        vp = cpsum.tile([128, D], F32, tag="chain", name="vp")
                        for ko in range(KO):
                            nc.tensor.matmul(vp, hf[:, ko, ts(t, 128)], wv[:, ko],
                                             start=ko == 0, stop=ko == KO - 1)
                        nc.scalar.copy(v_loc[:, t], vp)
                    nc.sync.dma_start(
                        out=v_local[:].rearrange("(t p) f -> p t f", p=128),
                        in_=v_loc)
                    v_full = dram.tile([CH * TLOC, D], BF16, tag="v_full",
                                       name="v_full")
                    nc.gpsimd.collective_compute(
                        "AllGather", ALU.bypass, replica_groups=groups,
                        ins=[v_local.opt()], outs=[v_full.opt()])

                    # ---- q (token-major) + 8/|q| fold + transpose to fm ----
                    qnf = qpool.tile([128, CH, TLOC], BF16, tag="qnf", name="qnf")
                    for t in range(QT):
                        qp = cpsum.tile([128, D], F32, tag="chain", name="qp")
                        for ko in range(KO):
                            nc.tensor.matmul(qp, hf[:, ko, ts(t, 128)], wq[:, ko],
                                             start=ko == 0, stop=ko == KO - 1)
                        sq = smalls.tile([128, D], F32, tag="sq", bufs=2, name="sq2")
                        nc.scalar.square(sq, qp)
                        ssq = smalls.tile([128, H], F32, tag="ssq", name="ssq2")
                        nc.vector.reduce_sum(ssq, sq.rearrange("p (h d) -> p h d", h=H),
                                             axis=AX)
                        nrm = smalls.tile([128, H], F32, tag="nrm", name="nrm2")
                        nc.scalar.activation(nrm, ssq, ACTF.Sqrt)
                        nc.vector.tensor_scalar_max(nrm, nrm, 1e-12)
                        rq = smalls.tile([128, H], F32, tag="rk", name="rq")
                        nc.vector.reciprocal(rq, nrm)
                        qn = qpool.tile([128, H, DH], BF16, tag="qn_tm", name="qn")
                        for hd in range(H):
                            nc.vector.tensor_scalar(
                                qn[:, hd], qp[:, ts(hd, DH)], rq[:, hd:hd + 1],
                                SCALE, ALU.mult, ALU.mult)
                        tp = tpsum.tile([64, H, 128], BF16, tag="tgrp", name="tpq")
                        for hd in range(H):
                            slot = (hd % 2) * 4 + hd // 2
                            nc.tensor.transpose(tp[:, slot], qn[:, hd], ident)
                        nc.vector.tensor_copy(
                            qnf[0:64, :, ts(t, 128)], tp[:, 0:4])
                        nc.vector.tensor_copy(
                            qnf[64:128, :, ts(t, 128)], tp[:, 4:8])

                    # ---- AG readback ----
                    knf = kvpool.tile([128, CH, CH, TLOC], BF16, tag="knf",
                                      name="knf")  # [2h, hp, c, l]
                    vfull = kvpool.tile([128, CH, QT, D], BF16, tag="vfull",
                                        name="vfull")  # [p, c, t, hd*dh]
                    for c in range(CH):
                        nc.sync.dma_start(
                            out=knf[:, :, c],
                            in_=k_full[c * TLOC:(c + 1) * TLOC]
                            .rearrange("(hp p) l -> p hp l", p=128))
                        nc.sync.dma_start(
                            out=vfull[:, c],
                            in_=v_full[c * TLOC:(c + 1) * TLOC]
                            .rearrange("(t p) f -> p t f", p=128))

                    # ---- HAM keep-warm filler ----
                    # Dependency-free matmuls that drain on the PE during the
                    # AllGather wait so the array re-enters attention at
                    # K=8/8 instead of half clock. ~20us of filler, consumed
                    # inside a 35-60us gap.
                    warm = tpsum.tile([128, KO, 128], F32, tag="tgrp",
                                      name="warm")
                    for wi in range(220):
                        nc.tensor.matmul(warm[:, wi % KO], identf,
                                         identf[:, 0:128], start=True,
                                         stop=True)

                    # ---- attention (transpose-free, key-major scores) ----
                    # head pair hp: head 2hp on array half 0, head 2hp+1 on half 1.
                    # s01 holds both halves' scores (one batched exp per block).
                    # v-MM of half 0 -> u[0:64] (col grp 0-1), its d-MM -> dps[96]
                    # (col grp 3, concurrent); half 1: u[64:128] / dps[32].
                    # Emission is software-pipelined: PV of block i issues after
                    # the scores of block i+1 so the PE never head-of-line blocks
                    # on the scalar-engine exp.
                    of = ofpool.tile([128, KO, TLOC], BF16, tag="of", name="of")
                    for hp in range(CH):
                        u = cpsum.tile([128, TLOC], F32, tag="chain", name="u")
                        dps = apsum.tile([128, TLOC], F32, tag="dps", name="dps")

                        def emit_pv(p01, qs, span, c, idx, first, stop):
                            nc.tensor.matmul(
                                u[0:64, qs:], vfull[:, c, idx, ts(2 * hp, DH)],
                                p01[:, 0, :span], start=first, stop=stop,
                                skip_group_check=True)
                            nc.tensor.matmul(
                                dps[96:97, qs:], ones1, p01[:, 0, :span],
                                start=first, stop=stop, skip_group_check=True,
                                tile_position=(0, 96))
                            nc.tensor.matmul(
                                u[64:128, qs:], vfull[:, c, idx,
                                                      ts(2 * hp + 1, DH)],
                                p01[:, 1, :span], start=first, stop=stop,
                                skip_group_check=True)
                            nc.tensor.matmul(
                                dps[32:33, qs:], ones1, p01[:, 1, :span],
                                start=first, stop=stop, skip_group_check=True,
                                tile_position=(0, 32))

                        prev = None
                        first = True
                        for idx in range(QT):
                            qs = idx * 128
                            span = TLOC - qs
                            for c in range(CH):
                                s01 = spsum.tile([128, 2, TLOC], F32, tag="s",
                                                 name="s01")
                                nc.tensor.matmul(
                                    s01[:, 0, :span], knf[0:64, hp, c, ts(idx, 128)],
                                    qnf[0:64, hp, qs:], start=True, stop=True)
                                nc.tensor.matmul(
                                    s01[:, 1, :span],
                                    knf[64:128, hp, c, ts(idx, 128)],
                                    qnf[64:128, hp, qs:], start=True, stop=True)
                                p01 = ppool.tile([128, 2, TLOC], BF16, tag="p",
                                                 name="p01")
                                nc.scalar.activation(p01[:, :, :span],
                                                     s01[:, :, :span], ACTF.Exp)
                                nc.vector.tensor_mul(p01[:, 0, 0:128],
                                                     p01[:, 0, 0:128],
                                                     masks_sb[:, c])
                                nc.vector.tensor_mul(p01[:, 1, 0:128],
                                                     p01[:, 1, 0:128],
                                                     masks_sb[:, c])
                                if prev is not None:
                                    emit_pv(*prev, first=first, stop=False)
                                    first = False
                                prev = (p01, qs, span, c, idx)
                        emit_pv(*prev, first=first, stop=True)
                        # softmax denominators: 1/d on DVE (PSUM->SBUF), broadcast
                        # to 64 lanes via a K=1 PE matmul, multiply into of.
                        rd = smalls.tile([128, TLOC], F32, tag="rd", bufs=2,
                                         name="rd")
                        nc.vector.reciprocal(rd[96:97], dps[96:97])
                        nc.vector.reciprocal(rd[32:33], dps[32:33])
                        bc = spsum.tile([128, 2, TLOC], F32, tag="s", name="bc")
                        nc.tensor.matmul(bc[0:64, 0], e64[96:97], rd[96:97],
                                         start=True, stop=True,
                                         tile_position=(96, 0))
                        nc.tensor.matmul(bc[64:128, 0], e64[32:33], rd[32:33],
                                         start=True, stop=True,
                                         tile_position=(32, 64))
                        nc.scalar.copy(of[0:64, hp], u[0:64])
                        nc.scalar.copy(of[64:128, hp], u[64:128])
                        nc.vector.tensor_mul(of[0:64, hp], of[0:64, hp],
                                             bc[0:64, 0])
                        nc.vector.tensor_mul(of[64:128, hp], of[64:128, hp],
                                             bc[64:128, 0])

                    # ---- Wo + residual + LN1 ----
                    for t in range(QT):
                        ap_ = cpsum.tile([128, D], F32, tag="chain", name="ap_")
                        for ko in range(KO):
                            nc.tensor.matmul(ap_, of[:, ko, ts(t, 128)], wo[:, ko],
                                             start=ko == 0, stop=ko == KO - 1)
                        nc.vector.tensor_scalar_mul(h[:, t], h[:, t], RES_SCALE)
                        nc.vector.tensor_add(h[:, t], h[:, t], ap_)
                        st = smalls.tile([128, 6], F32, tag="st", name="st")
                        nc.vector.bn_stats(st, h[:, t])
                        mv = smalls.tile([128, 2], F32, tag="mv", name="mv")
                        nc.vector.bn_aggr(mv, st)
                        sd = smalls.tile([128, 1], F32, tag="sd", name="sd")
                        nc.scalar.activation(sd, mv[:, 1:2], ACTF.Sqrt, bias=eps_t)
                        rstd = smalls.tile([128, 1], F32, tag="rstd", name="rstd")
                        nc.vector.reciprocal(rstd, sd)
                        nc.vector.tensor_scalar(h[:, t], h[:, t], mv[:, 0:1], rstd,
                                                ALU.subtract, ALU.mult)
                        nc.vector.tensor_mul(h[:, t], h[:, t], gb1[:, 0])
                        nc.vector.tensor_add(h[:, t], h[:, t], gb1[:, 1])

                    # ---- FFN ----
                    transpose_to_fm(h)
                    gf = gfpool.tile([128, MO, TLOC], BF16, tag="gf", name="gf")
                    for mo in range(MO):
                        up = cpsum.tile([128, TLOC], F32, tag="chain", name="up")
                        for ko in range(KO):
                            nc.tensor.matmul(up, w1[:, ko, ts(mo, 128)],
                                             hf[:, ko], start=ko == 0, stop=ko == KO - 1)
                        nc.scalar.activation(gf[:, mo], up, ACTF.Gelu)
                    for t in range(QT):
                        fp = cpsum.tile([128, D], F32, tag="chain", name="fp")
                        for mo in range(MO):
                            nc.tensor.matmul(fp, gf[:, mo, ts(t, 128)], w2[:, mo],
                                             start=mo == 0, stop=mo == MO - 1)
                        nc.vector.tensor_scalar_mul(h[:, t], h[:, t], RES_SCALE)
                        nc.vector.tensor_add(h[:, t], h[:, t], fp)
                        st = smalls.tile([128, 6], F32, tag="st", name="st2")
                        nc.vector.bn_stats(st, h[:, t])
                        mv = smalls.tile([128, 2], F32, tag="mv", name="mv2")
                        nc.vector.bn_aggr(mv, st)
                        sd = smalls.tile([128, 1], F32, tag="sd", name="sd2")
                        nc.scalar.activation(sd, mv[:, 1:2], ACTF.Sqrt, bias=eps_t)
                        rstd = smalls.tile([128, 1], F32, tag="rstd", name="rstd2")
                        nc.vector.reciprocal(rstd, sd)
                        nc.vector.tensor_scalar(h[:, t], h[:, t], mv[:, 0:1], rstd,
                                                ALU.subtract, ALU.mult)
                        nc.vector.tensor_mul(h[:, t], h[:, t], gb2[:, 0])
                        nc.vector.tensor_add(h[:, t], h[:, t], gb2[:, 1])

            # ---- logits ----
            with contextlib.ExitStack() as lg:
                wlpool = lg.enter_context(tc.tile_pool(name="wlpool", bufs=3))
                zsb = lg.enter_context(tc.tile_pool(name="zsb", bufs=4))
                zpsum = lg.enter_context(tc.tile_pool(name="zpsum", bufs=6, space="PSUM"))
                tpsum = lg.enter_context(tc.tile_pool(name="tps2", bufs=2, space="PSUM"))
                transpose_to_fm(h)
                nsub, ncols = NSUB
                for chk in range(NCHUNK):
                    wl = wlpool.tile([128, KO, NCOLS], BF16, tag="wl", name="wl")
                    half_c = NCOLS // 2
                    for hh in range(2):
                        nc.sync.dma_start(
                            out=wl[:, :, hh * half_c:(hh + 1) * half_c],
                            in_=wl_p[:, chk * NCOLS + hh * half_c:
                                     chk * NCOLS + (hh + 1) * half_c]
                            .rearrange("(ko ki) f -> ki ko f", ki=128))
                    for t in range(QT):
                        for nn in range(nsub):
                            zp = zpsum.tile([128, ncols], F32, tag="z", name="zp")
                            for ko in range(KO):
                                nc.tensor.matmul(
                                    zp, hf[:, ko, ts(t, 128)],
                                    wl[:, ko, ts(nn, ncols)],
                                    start=ko == 0, stop=ko == KO - 1)
                            zs = zsb.tile([128, ncols], BF16, tag="zs", name="zs")
                            if (t * nsub + nn) % 2 == 0:
                                nc.scalar.copy(zs, zp)
                            else:
                                nc.vector.tensor_copy(zs, zp)
                            nc.sync.dma_start(
                                out=out_p[ts(t, 128),
                                          chk * NCOLS + nn * ncols:
                                          chk * NCOLS + (nn + 1) * ncols],
                                in_=zs)
    if not nc.is_finalized():
        nc.finalize()
    return nc


_CACHE = {}


def _prep(inputs):
    bf = lambda a: np.ascontiguousarray(np.asarray(a)).astype(ml_dtypes.bfloat16)
    f32 = lambda a: np.ascontiguousarray(np.asarray(a, dtype=np.float32))
    x = np.asarray(inputs["x"])
    h0 = np.asarray(inputs["token_emb"])[x] + np.asarray(inputs["pos_emb"])[:N]
    h0 = h0.astype(np.float32)

    shared = dict(
        wq=bf(inputs["Wq"]), wk=bf(inputs["Wk"]), wv=bf(inputs["Wv"]),
        wo=bf(inputs["Wo"]), w1=bf(inputs["W1"]), w2=bf(inputs["W2"]),
        ln1g=f32(inputs["ln1_g"]), ln1b=f32(inputs["ln1_b"]),
        ln2g=f32(inputs["ln2_g"]), ln2b=f32(inputs["ln2_b"]),
        wl=bf(inputs["Wlogits"]),
    )
    j = np.arange(128)[:, None]   # partition: local key index within block
    m = np.arange(128)[None, :]   # free: local query index within block
    in_maps = []
    for core in range(8):
        b, r = core // 4, core % 4
        # transposed causal masks: maskT[c][k_loc, q_loc] = 4*k+c <= 4*q+r
        masks = np.stack([
            np.where(4 * j + c <= 4 * m + r, 1.0, 0.0)
            for c in range(CH)]).astype(ml_dtypes.bfloat16)
        in_maps.append(dict(shared, h0=np.ascontiguousarray(h0[b, r::4]),
                            masks=masks))
    return in_maps


def _run(inputs, trace=False, **kw):
    from concourse.bass_utils import run_bass_kernel_spmd
    if "nc" not in _CACHE:
        _CACHE["nc"] = build_nc()
    nc = _CACHE["nc"]
    in_maps = _prep(inputs)
    res = run_bass_kernel_spmd(nc, in_maps, core_ids=list(range(8)),
                               trace=trace, **kw)
    out = np.zeros((B, N, V), np.float32)
    for core in range(8):
        b, r = core // 4, core % 4
        out[b, r::4] = res.results[core]["out"].astype(np.float32)
    return out, res


def kernel(**inputs):
    return _run(inputs, trace=False)[0]


# revision 17
# speedup vs baseline: 1.8979x; 1.0363x over previous
"""Trainium2 Bass kernel for nn_CosineSimCausalTransformer (B=2, N=2048, D=512,
V=32000, 6 layers, 8 heads, cosine-sim causal attention, exact-gelu FFN).

Sharding (8 NeuronCores): 2 batch groups x 4 ranks. Core c = (batch=c//4,
rank r=c%4) owns token rows r::4 of its batch (512 rows) -- token-interleaved
so every core has an IDENTICAL causal block structure (true SPMD). Per layer
the normalized K (feature-major) and V (token-major) of the 4 ranks are
AllGather'ed (single fused k+v collective) within each batch group.

Attention is transpose-free: scores are computed key-major (s^T = K_n Q_n^T),
exp'd straight into SBUF as the P^T operand of the P.V matmul (V stationary).
Softmax denominators come from a concurrent M=1 ones-matmul into a spare PSUM
col-group (array col-groups 3/1, disjoint from the V-matmul's); 1/d is
broadcast to 64 lanes with a K=1 PE matmul and multiplied into the staged
attention output. Score/PV emission is software-pipelined one block deep so
the in-order PE queue never head-of-line blocks on the scalar-engine exp.

All matmuls run in bf16 with fp32 PSUM accumulation. Softmax skips
max-subtraction (scores bounded to [-8, 8] by cosine-sim) and folds 8/|q|
into q.
"""
import os
import sys

sys.path.insert(0, "/opt/trn_rl_repo")

import numpy as np
import ml_dtypes

import concourse.bass as bass
import concourse.bacc as bacc_mod
import concourse.mybir as mybir
import concourse.tile as tile
from concourse.bass import ts
from concourse.masks import make_identity

F32 = mybir.dt.float32
BF16 = mybir.dt.bfloat16

B, N, D, V = 2, 2048, 512, 32000
DEPTH, H, DH, DFF = 6, 8, 64, 2048
SCALE = 8.0
RES_SCALE = float((2 * DEPTH) ** 0.25)
EPS_LN = 1e-5
TLOC = 512        # tokens per core
QT = 4            # 128-row token tiles per core
KO = 4            # D/128 contraction subtiles
CH = 4            # key chunks (ranks per group)
MO = DFF // 128   # 16
NCHUNK, NCOLS = 16, 2000   # Wlogits column streaming (16*2000 = 32000)
NSUB = 4, 500

AX = mybir.AxisListType.X
ALU = mybir.AluOpType
ACTF = mybir.ActivationFunctionType


def build_nc():
    nc = bacc_mod.Bacc()

    h0_p = nc.declare_dram_parameter("h0", [TLOC, D], F32, isOutput=False)
    mask_p = nc.declare_dram_parameter("masks", [CH, 128, 128], BF16, isOutput=False)
    wq_p = nc.declare_dram_parameter("wq", [DEPTH, D, D], BF16, isOutput=False)
    wk_p = nc.declare_dram_parameter("wk", [DEPTH, D, D], BF16, isOutput=False)
    wv_p = nc.declare_dram_parameter("wv", [DEPTH, D, D], BF16, isOutput=False)
    wo_p = nc.declare_dram_parameter("wo", [DEPTH, D, D], BF16, isOutput=False)
    w1_p = nc.declare_dram_parameter("w1", [DEPTH, D, DFF], BF16, isOutput=False)
    w2_p = nc.declare_dram_parameter("w2", [DEPTH, DFF, D], BF16, isOutput=False)
    ln1g_p = nc.declare_dram_parameter("ln1g", [DEPTH, D], F32, isOutput=False)
    ln1b_p = nc.declare_dram_parameter("ln1b", [DEPTH, D], F32, isOutput=False)
    ln2g_p = nc.declare_dram_parameter("ln2g", [DEPTH, D], F32, isOutput=False)
    ln2b_p = nc.declare_dram_parameter("ln2b", [DEPTH, D], F32, isOutput=False)
    wl_p = nc.declare_dram_parameter("wl", [D, V], BF16, isOutput=False)
    out_p = nc.declare_dram_parameter("out", [TLOC, V], BF16, isOutput=True)

    groups = [[0, 1, 2, 3], [4, 5, 6, 7]]

    def bcast_row(ap_1d, parts=128):
        # [D] dram AP -> [[0,parts], ...] broadcast read
        return bass.AP(tensor=ap_1d.tensor, offset=ap_1d.offset,
                       ap=[[0, parts]] + list(ap_1d.ap))

    with tile.TileContext(nc) as tc:
        import contextlib
        with contextlib.ExitStack() as top:
            const = top.enter_context(tc.tile_pool(name="const", bufs=1))
            hpool = top.enter_context(tc.tile_pool(name="hpool", bufs=1))
            hfpool = top.enter_context(tc.tile_pool(name="hfpool", bufs=1))

            ident = const.tile([128, 128], BF16, name="ident")
            make_identity(nc, ident)
            identf = const.tile([128, 128], F32, name="identf")
            make_identity(nc, identf)
            masks_sb = const.tile([128, CH, 128], BF16, name="masks_sb")
            eps_t = const.tile([128, 1], F32, name="eps_t")
            nc.vector.memset(eps_t, EPS_LN)
            ones1 = const.tile([128, 1], BF16, name="ones1")
            nc.vector.memset(ones1, 1.0)
            e64 = const.tile([128, 64], F32, name="e64")
            nc.vector.memset(e64, 1.0)
            nc.sync.dma_start(out=masks_sb, in_=mask_p[:].rearrange("c j m -> j c m"))

            h = hpool.tile([128, QT, D], F32, name="h")   # persistent token-major h
            nc.sync.dma_start(out=h, in_=h0_p[:].rearrange("(t p) d -> p t d", p=128))
            hf = hfpool.tile([128, KO, TLOC], BF16, name="hf")  # feature-major bf16 h

            def transpose_to_fm(src_f32):
                """src [128, QT, D] f32 -> hf [128, KO, TLOC] bf16 (hf[d%128, d//128, tok])

                f32 PE transpose; the PSUM->SBUF copy performs the bf16 cast."""
                for t in range(QT):
                    tp = tpsum.tile([128, KO, 128], F32, tag="tgrp", name="tp")
                    for ko in range(KO):
                        nc.tensor.transpose(tp[:, ko], src_f32[:, t, ts(ko, 128)],
                                            identf)
                    nc.vector.tensor_copy(hf[:, :, ts(t, 128)], tp)

            with contextlib.ExitStack() as lay:
                wpool = lay.enter_context(tc.tile_pool(name="wpool", bufs=2))
                wbig = lay.enter_context(tc.tile_pool(name="wbig", bufs=1))
                gbpool = lay.enter_context(tc.tile_pool(name="gbpool", bufs=1))
                kvpool = lay.enter_context(tc.tile_pool(name="kvpool", bufs=1))
                qpool = lay.enter_context(tc.tile_pool(name="qpool", bufs=1))
                ppool = lay.enter_context(tc.tile_pool(name="ppool", bufs=4))
                smalls = lay.enter_context(tc.tile_pool(name="smalls", bufs=4))
                ofpool = lay.enter_context(tc.tile_pool(name="ofpool", bufs=1))
                gfpool = lay.enter_context(tc.tile_pool(name="gfpool", bufs=1))
                dram = lay.enter_context(tc.tile_pool(name="dram", bufs=2, space="DRAM"))

                cpsum = lay.enter_context(tc.tile_pool(name="cpsum", bufs=2, space="PSUM"))
                spsum = lay.enter_context(tc.tile_pool(name="spsum", bufs=2, space="PSUM"))
                # tpsum double-buffers and also hosts the attention d-rows
                # (same byte size, disjoint phases) so dps is double-buffered
                # without a 9th bank: the next head pair's d-matmul no longer
                # stalls behind this pair's reciprocals.
                tpsum = lay.enter_context(tc.tile_pool(name="tpsum", bufs=2, space="PSUM"))

                for li in range(DEPTH):
                    # ---- weights ----
                    wq = wpool.tile([128, KO, D], BF16, tag="wq", name="wq")
                    wk = wpool.tile([128, KO, D], BF16, tag="wk", name="wk")
                    wv = wpool.tile([128, KO, D], BF16, tag="wv", name="wv")
                    wo = wpool.tile([128, KO, D], BF16, tag="wo", name="wo")
                    for wt, wp in ((wq, wq_p), (wk, wk_p), (wv, wv_p), (wo, wo_p)):
                        nc.sync.dma_start(
                            out=wt, in_=wp[li].rearrange("(ko ki) f -> ki ko f", ki=128))
                    w1 = wbig.tile([128, KO, DFF], BF16, tag="w1", name="w1")
                    nc.sync.dma_start(
                        out=w1, in_=w1_p[li].rearrange("(ko ki) f -> ki ko f", ki=128))
                    w2 = wbig.tile([128, MO, D], BF16, tag="w2", name="w2")
                    nc.sync.dma_start(
                        out=w2, in_=w2_p[li].rearrange("(mo ki) f -> ki mo f", ki=128))
                    gb1 = gbpool.tile([128, 2, D], F32, tag="gb1", name="gb1")
                    nc.sync.dma_start(out=gb1[:, 0], in_=bcast_row(ln1g_p[li]))
                    nc.sync.dma_start(out=gb1[:, 1], in_=bcast_row(ln1b_p[li]))
                    gb2 = gbpool.tile([128, 2, D], F32, tag="gb2", name="gb2")
                    nc.sync.dma_start(out=gb2[:, 0], in_=bcast_row(ln2g_p[li]))
                    nc.sync.dma_start(out=gb2[:, 1], in_=bcast_row(ln2b_p[li]))

                    # ---- h -> feature-major bf16 ----
                    transpose_to_fm(h)

                    # ---- k (token-major), normalize, transpose to fm ----
                    kv_local = dram.tile([2 * TLOC, D], BF16, tag="kv_local",
                                         name="kv_local")
                    knf_loc = qpool.tile([128, CH, TLOC], BF16, tag="knf_loc",
                                         name="knf_loc")
                    v_loc = qpool.tile([128, QT, D], BF16, tag="v_loc", name="v_loc")
                    for t in range(QT):
                        kp = cpsum.tile([128, D], F32, tag="chain", name="kp")
                        for ko in range(KO):
                            nc.tensor.matmul(kp, hf[:, ko, ts(t, 128)], wk[:, ko],
                                             start=ko == 0, stop=ko == KO - 1)
                        sq = smalls.tile([128, D], F32, tag="sq", bufs=2, name="sq")
                        nc.scalar.square(sq, kp)
                        ssq = smalls.tile([128, H], F32, tag="ssq", name="ssq")
                        nc.vector.reduce_sum(ssq, sq.rearrange("p (h d) -> p h d", h=H),
                                             axis=AX)
                        nrm = smalls.tile([128, H], F32, tag="nrm", name="nrm")
                        nc.scalar.activation(nrm, ssq, ACTF.Sqrt)
                        nc.vector.tensor_scalar_max(nrm, nrm, 1e-12)
                        rk = smalls.tile([128, H], F32, tag="rk", name="rk")
                        nc.vector.reciprocal(rk, nrm)
                        kn = qpool.tile([128, H, DH], BF16, tag="kn_tm", name="kn")
                        for hd in range(H):
                            nc.vector.tensor_scalar_mul(
                                kn[:, hd], kp[:, ts(hd, DH)], rk[:, hd:hd + 1])
                        tp = tpsum.tile([64, H, 128], BF16, tag="tgrp", name="tpk")
                        for hd in range(H):
                            slot = (hd % 2) * 4 + hd // 2
                            nc.tensor.transpose(tp[:, slot], kn[:, hd], ident)
                        nc.vector.tensor_copy(
                            knf_loc[0:64, :, ts(t, 128)], tp[:, 0:4])
                        nc.vector.tensor_copy(
                            knf_loc[64:128, :, ts(t, 128)], tp[:, 4:8])
                    nc.sync.dma_start(
                        out=kv_local[0:TLOC].rearrange("(hp p) l -> p hp l", p=128),
                        in_=knf_loc)

                    # ---- v (token-major) ----
                    for t in range(QT):
                        vp = cpsum.tile([128, D], F32, tag="chain", name="vp")
                        for ko in range(KO):
                            nc.tensor.matmul(vp, hf[:, ko, ts(t, 128)], wv[:, ko],
                                             start=ko == 0, stop=ko == KO - 1)
                        nc.scalar.copy(v_loc[:, t], vp)
                    nc.sync.dma_start(
                        out=kv_local[TLOC:2 * TLOC].rearrange("(t p) f -> p t f", p=128),
                        in_=v_loc)

                    # ---- fused k+v AllGather ----
                    kv_full = dram.tile([CH * 2 * TLOC, D], BF16, tag="kv_full",
                                        name="kv_full")
                    nc.gpsimd.collective_compute(
                        "AllGather", ALU.bypass, replica_groups=groups,
                        ins=[kv_local.opt()], outs=[kv_full.opt()])

                    # ---- q (token-major) + 8/|q| fold + transpose to fm ----
                    qnf = qpool.tile([128, CH, TLOC], BF16, tag="qnf", name="qnf")
                    for t in range(QT):
                        qp = cpsum.tile([128, D], F32, tag="chain", name="qp")
                        for ko in range(KO):
                            nc.tensor.matmul(qp, hf[:, ko, ts(t, 128)], wq[:, ko],
                                             start=ko == 0, stop=ko == KO - 1)
                        sq = smalls.tile([128, D], F32, tag="sq", bufs=2, name="sq2")
                        nc.scalar.square(sq, qp)
                        ssq = smalls.tile([128, H], F32, tag="ssq", name="ssq2")
                        nc.vector.reduce_sum(ssq, sq.rearrange("p (h d) -> p h d", h=H),
                                             axis=AX)
                        nrm = smalls.tile([128, H], F32, tag="nrm", name="nrm2")
                        nc.scalar.activation(nrm, ssq, ACTF.Sqrt)
                        nc.vector.tensor_scalar_max(nrm, nrm, 1e-12)
                        rq = smalls.tile([128, H], F32, tag="rk", name="rq")
                        nc.vector.reciprocal(rq, nrm)
                        qn = qpool.tile([128, H, DH], BF16, tag="qn_tm", name="qn")
                        for hd in range(H):
                            nc.vector.tensor_scalar(
                                qn[:, hd], qp[:, ts(hd, DH)], rq[:, hd:hd + 1],
                                SCALE, ALU.mult, ALU.mult)
                        tp = tpsum.tile([64, H, 128], BF16, tag="tgrp", name="tpq")
                        for hd in range(H):
                            slot = (hd % 2) * 4 + hd // 2
                            nc.tensor.transpose(tp[:, slot], qn[:, hd], ident)
                        nc.vector.tensor_copy(
                            qnf[0:64, :, ts(t, 128)], tp[:, 0:4])
                        nc.vector.tensor_copy(
                            qnf[64:128, :, ts(t, 128)], tp[:, 4:8])

                    # ---- AG readback ----
                    knf = kvpool.tile([128, CH, CH, TLOC], BF16, tag="knf",
                                      name="knf")  # [2h, hp, c, l]
                    vfull = kvpool.tile([128, CH, QT, D], BF16, tag="vfull",
                                        name="vfull")  # [p, c, t, hd*dh]
                    for c in range(CH):
                        nc.sync.dma_start(
                            out=knf[:, :, c],
                            in_=kv_full[c * 2 * TLOC:c * 2 * TLOC + TLOC]
                            .rearrange("(hp p) l -> p hp l", p=128))
                        nc.sync.dma_start(
                            out=vfull[:, c],
                            in_=kv_full[c * 2 * TLOC + TLOC:(c + 1) * 2 * TLOC]
                            .rearrange("(t p) f -> p t f", p=128))

                    # ---- attention (transpose-free, key-major scores) ----
                    # head pair hp: head 2hp on array half 0, head 2hp+1 on half 1.
                    # s01 holds both halves' scores (one batched exp per block).
                    # v-MM of half 0 -> u[0:64] (col grp 0-1), its d-MM -> dps[96]
                    # (col grp 3, concurrent); half 1: u[64:128] / dps[32].
                    # Emission is software-pipelined: PV of block i issues after
                    # the scores of block i+1 so the PE never head-of-line blocks
                    # on the scalar-engine exp.
                    of = ofpool.tile([128, KO, TLOC], BF16, tag="of", name="of")
                    for hp in range(CH):
                        u = cpsum.tile([128, TLOC], F32, tag="chain", name="u")
                        dps = tpsum.tile([128, TLOC], F32, tag="tgrp", name="dps")

                        def emit_pv(p01, qs, span, c, idx, first, stop):
                            nc.tensor.matmul(
                                u[0:64, qs:], vfull[:, c, idx, ts(2 * hp, DH)],
                                p01[:, 0, :span], start=first, stop=stop,
                                skip_group_check=True)
                            nc.tensor.matmul(
                                dps[96:97, qs:], ones1, p01[:, 0, :span],
                                start=first, stop=stop, skip_group_check=True,
                                tile_position=(0, 96))
                            nc.tensor.matmul(
                                u[64:128, qs:], vfull[:, c, idx,
                                                      ts(2 * hp + 1, DH)],
                                p01[:, 1, :span], start=first, stop=stop,
                                skip_group_check=True)
                            nc.tensor.matmul(
                                dps[32:33, qs:], ones1, p01[:, 1, :span],
                                start=first, stop=stop, skip_group_check=True,
                                tile_position=(0, 32))

                        prev = None
                        first = True
                        for idx in range(QT):
                            qs = idx * 128
                            span = TLOC - qs
                            for c in range(CH):
                                s01 = spsum.tile([128, 2, TLOC], F32, tag="s",
                                                 name="s01")
                                nc.tensor.matmul(
                                    s01[:, 0, :span], knf[0:64, hp, c, ts(idx, 128)],
                                    qnf[0:64, hp, qs:], start=True, stop=True)
                                nc.tensor.matmul(
                                    s01[:, 1, :span],
                                    knf[64:128, hp, c, ts(idx, 128)],
                                    qnf[64:128, hp, qs:], start=True, stop=True)
                                p01 = ppool.tile([128, 2, TLOC], BF16, tag="p",
                                                 name="p01")
                                nc.scalar.activation(p01[:, :, :span],
                                                     s01[:, :, :span], ACTF.Exp)
                                nc.vector.tensor_mul(p01[:, 0, 0:128],
                                                     p01[:, 0, 0:128],
                                                     masks_sb[:, c])
                                nc.vector.tensor_mul(p01[:, 1, 0:128],
                                                     p01[:, 1, 0:128],
                                                     masks_sb[:, c])
                                if prev is not None:
                                    emit_pv(*prev, first=first, stop=False)
                                    first = False
                                prev = (p01, qs, span, c, idx)
                        emit_pv(*prev, first=first, stop=True)
                        # softmax denominators: 1/d on DVE (PSUM->SBUF), broadcast
                        # to 64 lanes via a K=1 PE matmul, multiply into of.
                        rd = smalls.tile([128, TLOC], F32, tag="rd", bufs=2,
                                         name="rd")
                        nc.vector.reciprocal(rd[96:97], dps[96:97])
                        nc.vector.reciprocal(rd[32:33], dps[32:33])
                        bc = spsum.tile([128, 2, TLOC], F32, tag="s", name="bc")
                        nc.tensor.matmul(bc[0:64, 0], e64[96:97], rd[96:97],
                                         start=True, stop=True,
                                         tile_position=(96, 0))
                        nc.tensor.matmul(bc[64:128, 0], e64[32:33], rd[32:33],
                                         start=True, stop=True,
                                         tile_position=(32, 64))
                        nc.scalar.copy(of[0:64, hp], u[0:64])
                        nc.scalar.copy(of[64:128, hp], u[64:128])
                        nc.vector.tensor_mul(of[0:64, hp], of[0:64, hp],
                                             bc[0:64, 0])
                        nc.vector.tensor_mul(of[64:128, hp], of[64:128, hp],
                                             bc[64:128, 0])

                    # ---- Wo + residual + LN1 ----
                    for t in range(QT):
                        ap_ = cpsum.tile([128, D], F32, tag="chain", name="ap_")
                        for ko in range(KO):
                            nc.tensor.matmul(ap_, of[:, ko, ts(t, 128)], wo[:, ko],
                                             start=ko == 0, stop=ko == KO - 1)
                        nc.vector.tensor_scalar_mul(h[:, t], h[:, t], RES_SCALE)
                        nc.vector.tensor_add(h[:, t], h[:, t], ap_)
                        st = smalls.tile([128, 6], F32, tag="st", name="st")
                        nc.vector.bn_stats(st, h[:, t])
                        mv = smalls.tile([128, 2], F32, tag="mv", name="mv")
                        nc.vector.bn_aggr(mv, st)
                        sd = smalls.tile([128, 1], F32, tag="sd", name="sd")
                        nc.scalar.activation(sd, mv[:, 1:2], ACTF.Sqrt, bias=eps_t)
                        rstd = smalls.tile([128, 1], F32, tag="rstd", name="rstd")
                        nc.vector.reciprocal(rstd, sd)
                        nc.vector.tensor_scalar(h[:, t], h[:, t], mv[:, 0:1], rstd,
                                                ALU.subtract, ALU.mult)
                        nc.vector.tensor_mul(h[:, t], h[:, t], gb1[:, 0])
                        nc.vector.tensor_add(h[:, t], h[:, t], gb1[:, 1])

                    # ---- FFN ----
                    transpose_to_fm(h)
                    gf = gfpool.tile([128, MO, TLOC], BF16, tag="gf", name="gf")
                    for mo in range(MO):
                        up = cpsum.tile([128, TLOC], F32, tag="chain", name="up")
                        for ko in range(KO):
                            nc.tensor.matmul(up, w1[:, ko, ts(mo, 128)],
                                             hf[:, ko], start=ko == 0, stop=ko == KO - 1)
                        nc.scalar.activation(gf[:, mo], up, ACTF.Gelu)
                    for t in range(QT):
                        fp = cpsum.tile([128, D], F32, tag="chain", name="fp")
                        for mo in range(MO):
                            nc.tensor.matmul(fp, gf[:, mo, ts(t, 128)], w2[:, mo],
                                             start=mo == 0, stop=mo == MO - 1)
                        nc.vector.tensor_scalar_mul(h[:, t], h[:, t], RES_SCALE)
                        nc.vector.tensor_add(h[:, t], h[:, t], fp)
                        st = smalls.tile([128, 6], F32, tag="st", name="st2")
                        nc.vector.bn_stats(st, h[:, t])
                        mv = smalls.tile([128, 2], F32, tag="mv", name="mv2")
                        nc.vector.bn_aggr(mv, st)
                        sd = smalls.tile([128, 1], F32, tag="sd", name="sd2")
                        nc.scalar.activation(sd, mv[:, 1:2], ACTF.Sqrt, bias=eps_t)
                        rstd = smalls.tile([128, 1], F32, tag="rstd", name="rstd2")
                        nc.vector.reciprocal(rstd, sd)
                        nc.vector.tensor_scalar(h[:, t], h[:, t], mv[:, 0:1], rstd,
                                                ALU.subtract, ALU.mult)
                        nc.vector.tensor_mul(h[:, t], h[:, t], gb2[:, 0])
                        nc.vector.tensor_add(h[:, t], h[:, t], gb2[:, 1])

            # ---- logits ----
            with contextlib.ExitStack() as lg:
                wlpool = lg.enter_context(tc.tile_pool(name="wlpool", bufs=3))
                zsb = lg.enter_context(tc.tile_pool(name="zsb", bufs=4))
                zpsum = lg.enter_context(tc.tile_pool(name="zpsum", bufs=6, space="PSUM"))
                tpsum = lg.enter_context(tc.tile_pool(name="tps2", bufs=2, space="PSUM"))
                transpose_to_fm(h)
                nsub, ncols = NSUB
                for chk in range(NCHUNK):
                    wl = wlpool.tile([128, KO, NCOLS], BF16, tag="wl", name="wl")
                    half_c = NCOLS // 2
                    for hh in range(2):
                        nc.sync.dma_start(
                            out=wl[:, :, hh * half_c:(hh + 1) * half_c],
                            in_=wl_p[:, chk * NCOLS + hh * half_c:
                                     chk * NCOLS + (hh + 1) * half_c]
                            .rearrange("(ko ki) f -> ki ko f", ki=128))
                    for t in range(QT):
                        for nn in range(nsub):
                            zp = zpsum.tile([128, ncols], F32, tag="z", name="zp")
                            for ko in range(KO):
                                nc.tensor.matmul(
                                    zp, hf[:, ko, ts(t, 128)],
                                    wl[:, ko, ts(nn, ncols)],
                                    start=ko == 0, stop=ko == KO - 1)
                            zs = zsb.tile([128, ncols], BF16, tag="zs", name="zs")
                            if (t * nsub + nn) % 2 == 0:
                                nc.scalar.copy(zs, zp)
                            else:
                                nc.vector.tensor_copy(zs, zp)
                            nc.sync.dma_start(
                                out=out_p[ts(t, 128),
                                          chk * NCOLS + nn * ncols:
                                          chk * NCOLS + (nn + 1) * ncols],
                                in_=zs)
    if not nc.is_finalized():
        nc.finalize()
    return nc


_CACHE = {}


def _prep(inputs):
    bf = lambda a: np.ascontiguousarray(np.asarray(a)).astype(ml_dtypes.bfloat16)
    f32 = lambda a: np.ascontiguousarray(np.asarray(a, dtype=np.float32))
    x = np.asarray(inputs["x"])
    h0 = np.asarray(inputs["token_emb"])[x] + np.asarray(inputs["pos_emb"])[:N]
    h0 = h0.astype(np.float32)

    shared = dict(
        wq=bf(inputs["Wq"]), wk=bf(inputs["Wk"]), wv=bf(inputs["Wv"]),
        wo=bf(inputs["Wo"]), w1=bf(inputs["W1"]), w2=bf(inputs["W2"]),
        ln1g=f32(inputs["ln1_g"]), ln1b=f32(inputs["ln1_b"]),
        ln2g=f32(inputs["ln2_g"]), ln2b=f32(inputs["ln2_b"]),
        wl=bf(inputs["Wlogits"]),
    )
    j = np.arange(128)[:, None]   # partition: local key index within block
    m = np.arange(128)[None, :]   # free: local query index within block
    in_maps = []
    for core in range(8):
        b, r = core // 4, core % 4
        # transposed causal masks: maskT[c][k_loc, q_loc] = 4*k+c <= 4*q+r
        masks = np.stack([
            np.where(4 * j + c <= 4 * m + r, 1.0, 0.0)
            for c in range(CH)]).astype(ml_dtypes.bfloat16)
        in_maps.append(dict(shared, h0=np.ascontiguousarray(h0[b, r::4]),
                            masks=masks))
    return in_maps


def _run(inputs, trace=False, **kw):
    from concourse.bass_utils import run_bass_kernel_spmd
    if "nc" not in _CACHE:
        _CACHE["nc"] = build_nc()
    nc = _CACHE["nc"]
    in_maps = _prep(inputs)
    res = run_bass_kernel_spmd(nc, in_maps, core_ids=list(range(8)),
                               trace=trace, **kw)
    out = np.zeros((B, N, V), np.float32)
    for core in range(8):
        b, r = core // 4, core % 4
        out[b, r::4] = res.results[core]["out"].astype(np.float32)
    return out, res


def kernel(**inputs):
    return _run(inputs, trace=False)[0]
